# revision 10
# baseline (speedup 1.0000x reference)
"""Trainium2 Bass/Tile kernel for nn_Kernel_15812022909139887089.

Data-parallel over batch n: 8 NeuronCores, one batch element per core,
all params replicated. Each core computes the full fused graph for its n.

Graph (per n), C=256, H=W=56, S=3136, c8=32:
  t3   = (p2*x)^2                         [C,S]
  t5   = softmax_h(roll(t3,+1h,-1w))      [C,S]  (roll fused into exp read APs)
  t7   = conv1x1(unfold33_dil3(x), w7)    [C,S]  (9 shifted matmuls, PSUM acc)
  t8   = t5 @ t3^T / 56                   [C,C]  (via PE-transposed t5T,t3T)
  t11  = sum_b p11_b*(p9*conv1x1(t3,w6))_b  [1,S]
  t15  = conv1x1(roll(x,+1h), w15);  t17 = t3-t15;  t18 = t7*t17
  t16  = sum_b t8[b,c] p16[b,s]           [S,C]  (natural [s,c] layout)
  t19T = sum_s t16[s,d] t17[c,s] / 56     [d,c]  (rhs = PE-transposed t17T)
  t10  = max(t5,t7); depthwise t12 folded into t20 matmuls:
  A_i  = -s_c*w12_i (*) t19T;  t20 = s_c*S(x)t11 + sum_i A_i^T @ shift_h(t10, 2(i-1))
  out  = t20 + t18
"""

import os
import sys

for _p in ("/opt/trn_rl_repo", "/opt/pypackages"):
    if os.path.isdir(_p) and _p not in sys.path:
        sys.path.append(_p)

import math
import numpy as np

import concourse.bass as bass
import concourse.tile as tile
from concourse import bacc, mybir, masks
from concourse import bass_utils

F32 = mybir.dt.float32

N, C, H, W = 8, 256, 56, 56
S = H * W            # 3136
C8 = C // 8          # 32
NCORES = 8
PART = 128
NCC = C // PART      # 2 channel chunks
NSC = (S + PART - 1) // PART   # 25 s-chunks (24 full + one of 64)
S_HW = 1.0 / math.sqrt(S)      # 1/56
S_C = 1.0 / math.sqrt(C)       # 1/16

AF = mybir.ActivationFunctionType
AX = mybir.AxisListType


def _sc_rng(sc):
    lo = sc * PART
    return lo, min(PART, S - lo)


def build_module():
    nc = bacc.Bacc(
        "TRN2",
        target_bir_lowering=False,
        debug=False,
        num_devices=NCORES,
    )

    x_d = nc.dram_tensor("x", [C, S], F32, kind="ExternalInput")
    p2_d = nc.dram_tensor("p2s", [C, S], F32, kind="ExternalInput")
    w7_d = nc.dram_tensor("w7r", [9 * C, C], F32, kind="ExternalInput")
    w6_d = nc.dram_tensor("w6T", [C, C8], F32, kind="ExternalInput")
    p9_d = nc.dram_tensor("p9s", [C8, S], F32, kind="ExternalInput")
    p11_d = nc.dram_tensor("p11", [C8, 1], F32, kind="ExternalInput")
    w12_d = nc.dram_tensor("w12n", [C, 3], F32, kind="ExternalInput")
    w15_d = nc.dram_tensor("w15T", [C, C], F32, kind="ExternalInput")
    p16_d = nc.dram_tensor("p16", [C, S], F32, kind="ExternalInput")
    out_d = nc.dram_tensor("out", [C, S], F32, kind="ExternalOutput")

    with tile.TileContext(nc) as tc:
        _emit(nc, tc, x_d, p2_d, w7_d, w6_d, p9_d, p11_d, w12_d, w15_d, p16_d, out_d)

    nc.compile()
    return nc


class _Pools:
    """Pools with manual close for phase-bounded SBUF lifetimes."""

    def __init__(self, tc):
        self.tc = tc
        self._cms = {}

    def open(self, name, **kw):
        cm = self.tc.tile_pool(name=name, **kw)
        pool = cm.__enter__()
        self._cms[name] = cm
        return pool

    def close(self, name):
        self._cms.pop(name).__exit__(None, None, None)

    def close_all(self):
        while self._cms:
            name = next(reversed(self._cms))
            self.close(name)


def _emit(nc, tc, x_d, p2_d, w7_d, w6_d, p9_d, p11_d, w12_d, w15_d, p16_d, out_d):
    P = _Pools(tc)
    # LEFT side: long-lived
    const_pool = P.open("const", bufs=1, side="left")
    t5_pool = P.open("t5p", bufs=1, side="left")
    t7_pool = P.open("t7p", bufs=1, side="left")
    t17_pool = P.open("t17p", bufs=1, side="left")
    # RIGHT side: phase-scoped (strict LIFO)
    x_pool = P.open("xp", bufs=1, side="right")
    t3_pool = P.open("t3p", bufs=1, side="right")
    w7_pool = P.open("w7p", bufs=1, side="right")
    p2_pool = P.open("p2p", bufs=1, side="right")
    # PSUM
    ps_acc = P.open("ps_acc", bufs=1, space="PSUM")
    ps_mm = P.open("ps_mm", bufs=4, space="PSUM")
    ps_sm = P.open("ps_sm", bufs=1, space="PSUM")

    # ----------------- constants / params -----------------
    ident = const_pool.tile([PART, PART], F32, tag="ident", name="ident")
    masks.make_identity(nc, ident[:])
    ones = const_pool.tile([PART, 1], F32, tag="ones", name="ones")
    nc.vector.memset(ones[:], 1.0)

    WP = 62  # padded row width: 3 zero cols + 56 + 3 zero cols
    x_sb = [x_pool.tile([PART, H * WP], F32, tag=f"x{cc}", name=f"x{cc}") for cc in range(NCC)]

    def xv(cc):
        return x_sb[cc][:].rearrange("p (h w) -> p h w", w=WP)
    p2_sb = [p2_pool.tile([PART, S], F32, tag=f"p2{cc}", name=f"p2{cc}") for cc in range(NCC)]
    w15_sb = [const_pool.tile([PART, C], F32, tag=f"w15{cc}", name=f"w15{cc}") for cc in range(NCC)]
    w6_sb = [const_pool.tile([PART, C8], F32, tag=f"w6{cc}", name=f"w6{cc}") for cc in range(NCC)]
    w12_sb = [const_pool.tile([PART, 3], F32, tag=f"w12{cc}", name=f"w12{cc}") for cc in range(NCC)]
    p11_sb = const_pool.tile([C8, 1], F32, tag="p11", name="p11")
    w7_sb = [[w7_pool.tile([PART, C], F32, tag=f"w7_{ij}_{cc}", name=f"w7_{ij}_{cc}")
              for cc in range(NCC)] for ij in range(9)]

    x_dv = x_d.rearrange("(k p) s -> k p s", p=PART)
    p2_dv = p2_d.rearrange("(k p) s -> k p s", p=PART)
    p16_dv = p16_d.rearrange("(k p) s -> k p s", p=PART)
    w15_dv = w15_d.rearrange("(k p) o -> k p o", p=PART)
    w6_dv = w6_d.rearrange("(k p) o -> k p o", p=PART)
    w12_dv = w12_d.rearrange("(k p) o -> k p o", p=PART)
    w7_dv = w7_d.rearrange("(ij k p) o -> ij k p o", k=NCC, p=PART)

    # priority order: x first (t3+t7), then w7 (t7), then p2 (t3)
    for cc in range(NCC):
        nc.vector.memset(x_sb[cc][:], 0.0)
        nc.sync.dma_start(xv(cc)[:, :, 3:3 + W],
                          x_dv[cc].rearrange("p (h w) -> p h w", w=W))
    for ij in range(9):
        for cc in range(NCC):
            nc.sync.dma_start(w7_sb[ij][cc][:], w7_dv[ij, cc])
    for cc in range(NCC):
        nc.sync.dma_start(p2_sb[cc][:], p2_dv[cc])
        nc.sync.dma_start(w15_sb[cc][:], w15_dv[cc])
        nc.sync.dma_start(w6_sb[cc][:], w6_dv[cc])
        nc.sync.dma_start(w12_sb[cc][:], w12_dv[cc])
    nc.sync.dma_start(p11_sb[:], p11_d[:, :])

    # ----------------- t3 = (p2*x)^2 -----------------
    t3_sb = [t3_pool.tile([PART, S], F32, tag=f"t3{cc}", name=f"t3{cc}") for cc in range(NCC)]
    for cc in range(NCC):
        nc.vector.tensor_mul(t3_sb[cc][:].rearrange("p (h w) -> p h w", w=W),
                             xv(cc)[:, :, 3:3 + W],
                             p2_sb[cc][:].rearrange("p (h w) -> p h w", w=W))
        nc.scalar.activation(t3_sb[cc][:], t3_sb[cc][:], AF.Square)
    P.close("p2p")

    # ----------------- softmax (rolled) -> t5 -----------------
    t5_sb = [t5_pool.tile([PART, S], F32, tag=f"t5{cc}", name=f"t5{cc}") for cc in range(NCC)]
    for cc in range(NCC):
        ev = t5_sb[cc][:].rearrange("p (h w) -> p h w", h=H)
        tv = t3_sb[cc][:].rearrange("p (h w) -> p h w", h=H)
        # t4[c,h,w] = t3[c,(h-1)%H,(w+1)%W] ; E = exp(t4)
        nc.scalar.activation(ev[:, 1:, :W - 1], tv[:, :H - 1, 1:], AF.Exp)
        nc.scalar.activation(ev[:, 1:, W - 1:], tv[:, :H - 1, :1], AF.Exp)
        nc.scalar.activation(ev[:, :1, :W - 1], tv[:, H - 1:, 1:], AF.Exp)
        nc.scalar.activation(ev[:, :1, W - 1:], tv[:, H - 1:, :1], AF.Exp)
        d_t = const_pool.tile([PART, W], F32, tag=f"dsum{cc}", name=f"dsum{cc}")
        dinv_t = const_pool.tile([PART, W], F32, tag=f"dinv{cc}", name=f"dinv{cc}")
        ewh = t5_sb[cc][:].rearrange("p (h w) -> p w h", h=H)
        nc.vector.reduce_sum(d_t[:], ewh, axis=AX.X)
        nc.vector.reciprocal(dinv_t[:], d_t[:])
        dinv_b = dinv_t[:].unsqueeze(1).broadcast_to([PART, H, W])
        nc.vector.tensor_mul(ev, ev, dinv_b)

    # ----------------- t7: 3x3 dil-3 conv via 9 shifted matmuls -----------------
    t7_sb = [t7_pool.tile([PART, S], F32, tag=f"t7{cc}", name=f"t7{cc}") for cc in range(NCC)]
    SHIFT_ORDER = [(1, 1), (0, 0), (0, 1), (0, 2), (1, 0), (1, 2), (2, 0), (2, 1), (2, 2)]
    HCH = 8  # h rows per psum chunk -> N = 448
    NHC = H // HCH
    for mc in range(NCC):
        for hc in range(NHC):
            h0 = hc * HCH
            psum = ps_mm.tile([PART, HCH * W], F32, tag="mmbank", name="mmbank")
            for si, (i, j) in enumerate(SHIFT_ORDER):
                dh, dw = 3 * (i - 1), 3 * (j - 1)
                hlo = max(h0, -dh)
                hhi = min(h0 + HCH, H - dh)
                assert hlo < hhi
                ijk = i * 3 + j
                for cc in range(NCC):
                    out_ap = psum[:, (hlo - h0) * W:(hhi - h0) * W]
                    rhs_ap = xv(cc)[:, hlo + dh:hhi + dh, 3 + dw:3 + dw + W]
                    lhsT = w7_sb[ijk][cc][:, mc * PART:(mc + 1) * PART]
                    nc.tensor.matmul(
                        out_ap, lhsT, rhs_ap,
                        start=(si == 0 and cc == 0),
                        stop=(si == len(SHIFT_ORDER) - 1 and cc == NCC - 1),
                        skip_group_check=True,
                    )
            nc.scalar.copy(t7_sb[mc][:, h0 * W:(h0 + HCH) * W], psum[:])
    P.close("w7p")

    # ----------------- fused: t3T/t5T transposes + t8 accumulation -----------------
    # t8[c,d] = sum_s t5[c,s] t3[d,s] * s_hw, pipelined per 128-s-chunk
    tT_pool = P.open("tTp", bufs=3, side="right")
    t8_ps = [ps_acc.tile([PART, C], F32, tag=f"acc{mc}", name=f"t8acc{mc}")
             for mc in range(NCC)]

    def _transpose_pair(sc):
        lo, sz = _sc_rng(sc)
        slots = {}
        for ti, (src, nm) in enumerate(((t3_sb, "t3T"), (t5_sb, "t5T"))):
            psum = ps_mm.tile([PART, 2 * PART], F32, tag="mmbank", name="mmbank")
            for cc in range(NCC):
                nc.tensor.transpose(
                    psum[:sz, cc * PART:(cc + 1) * PART],
                    src[cc][:, lo:lo + sz], ident[:])
            slot = tT_pool.tile([PART, C], F32, tag=nm, name=nm)
            if ti == 0:
                nc.scalar.copy(slot[:sz, :], psum[:sz, :])
            else:
                nc.vector.tensor_copy(slot[:sz, :], psum[:sz, :])
            slots[nm] = slot
        return slots

    def _t8_mms(sc, slots):
        lo, sz = _sc_rng(sc)
        for mc in range(NCC):
            nc.tensor.matmul(
                t8_ps[mc][:, :],
                slots["t5T"][:sz, mc * PART:(mc + 1) * PART],
                slots["t3T"][:sz, :],
                start=(sc == 0), stop=(sc == NSC - 1),
            )

    prev = None
    for sc in range(NSC):
        cur = _transpose_pair(sc)
        if prev is not None:
            _t8_mms(sc - 1, prev)
        prev = cur
    _t8_mms(NSC - 1, prev)

    t8_sb = [const_pool.tile([PART, C], F32, tag=f"t8{mc}", name=f"t8{mc}")
             for mc in range(NCC)]
    for mc in range(NCC):
        nc.scalar.mul(t8_sb[mc][:], t8_ps[mc][:], S_HW)
    P.close("tTp")

    # ----------------- t6 -> t9 -> t11 -----------------
    p9_pool = P.open("p9p", bufs=1, side="right")
    p9_sb = p9_pool.tile([C8, S], F32, tag="p9", name="p9")
    nc.sync.dma_start(p9_sb[:], p9_d[:, :])
    t11_sb = const_pool.tile([1, S], F32, tag="t11", name="t11")
    for hc in range(7):
        n0 = hc * 448
        psum = ps_sm.tile([C8, 448], F32, tag="smbank", name="smbank")
        for cc in range(NCC):
            nc.tensor.matmul(
                psum[:, :], w6_sb[cc][:], t3_sb[cc][:, n0:n0 + 448],
                start=(cc == 0), stop=(cc == NCC - 1),
            )
        # t9 computed in place over p9
        nc.vector.tensor_mul(p9_sb[:, n0:n0 + 448], psum[:], p9_sb[:, n0:n0 + 448])
    t9_sb = p9_sb
    for k in range(7):
        n0 = k * 448
        psum = ps_sm.tile([1, 448], F32, tag="onebank", name="onebank")
        nc.tensor.matmul(psum[:, :], p11_sb[:], t9_sb[:, n0:n0 + 448],
                         start=True, stop=True)
        nc.scalar.copy(t11_sb[:, n0:n0 + 448], psum[:])
    P.close("p9p")

    # p16 load early for the later t16 phase (LEFT side, closes with t17p)
    p16_pool = P.open("p16p", bufs=1, side="left")
    p16_sb = [p16_pool.tile([PART, S], F32, tag=f"p16{cc}", name=f"p16{cc}")
              for cc in range(NCC)]
    for cc in range(NCC):
        nc.sync.dma_start(p16_sb[cc][:], p16_dv[cc])

    # ----------------- t15 -> t17 = t3 - t15 -----------------
    t17_sb = [t17_pool.tile([PART, S], F32, tag=f"t17{cc}", name=f"t17{cc}")
              for cc in range(NCC)]
    chunks = [(56 + 448 * k, 448) for k in range(6)] + [(2744, 392), (0, 56)]
    for mc in range(NCC):
        for (d0, ln) in chunks:
            s0 = d0 - 56 if d0 >= 56 else S - 56
            r0, nr = s0 // W, ln // W
            psum = ps_mm.tile([PART, 448], F32, tag="mmbank", name="mmbank")
            for cc in range(NCC):
                nc.tensor.matmul(
                    psum[:, :ln],
                    w15_sb[cc][:, mc * PART:(mc + 1) * PART],
                    xv(cc)[:, r0:r0 + nr, 3:3 + W],
                    start=(cc == 0), stop=(cc == NCC - 1),
                )
            nc.vector.tensor_sub(t17_sb[mc][:, d0:d0 + ln],
                                 t3_sb[mc][:, d0:d0 + ln], psum[:, :ln])
    P.close("t3p")
    P.close("xp")

    # ----------------- t10 = max(t5,t7) ; t18 = t7*t17 -----------------
    for cc in range(NCC):
        nc.vector.tensor_max(t5_sb[cc][:], t5_sb[cc][:], t7_sb[cc][:])
    t10_sb = t5_sb
    for cc in range(NCC):
        nc.vector.tensor_mul(t7_sb[cc][:], t7_sb[cc][:], t17_sb[cc][:])
    t18_sb = t7_sb

    # ----------------- fused: t16 + t17T + t19T accumulation -----------------
    # t16[s,c] = sum_b t8[b,c] p16[b,s] ; t19T[d,c] = sum_s t16[s,d] t17T[s,c] * s_hw
    t16_pool = P.open("t16p", bufs=3, side="right")
    t17T_pool = P.open("t17Tp", bufs=3, side="right")
    t19_ps = [ps_acc.tile([PART, C], F32, tag=f"acc{mc}", name=f"t19acc{mc}")
              for mc in range(NCC)]

    def _mk_t16_t17T(sc):
        lo, sz = _sc_rng(sc)
        ps16 = ps_mm.tile([PART, C], F32, tag="mmbank", name="mmbank")
        for kb in range(NCC):
            nc.tensor.matmul(
                ps16[:sz, :], p16_sb[kb][:, lo:lo + sz], t8_sb[kb][:],
                start=(kb == 0), stop=(kb == NCC - 1),
            )
        t16_t = t16_pool.tile([PART, C], F32, tag="t16s", name="t16s")
        nc.scalar.copy(t16_t[:sz, :], ps16[:sz, :])
        psT = ps_mm.tile([PART, 2 * PART], F32, tag="mmbank", name="mmbank")
        for cc in range(NCC):
            nc.tensor.transpose(
                psT[:sz, cc * PART:(cc + 1) * PART],
                t17_sb[cc][:, lo:lo + sz], ident[:])
        t17T_t = t17T_pool.tile([PART, C], F32, tag="t17Ts", name="t17Ts")
        nc.vector.tensor_copy(t17T_t[:sz, :], psT[:sz, :])
        return (t16_t, t17T_t)

    def _t19_mms(sc, pair):
        lo, sz = _sc_rng(sc)
        t16_t, t17T_t = pair
        for mc in range(NCC):
            nc.tensor.matmul(
                t19_ps[mc][:, :],
                t16_t[:sz, mc * PART:(mc + 1) * PART],
                t17T_t[:sz, :],
                start=(sc == 0), stop=(sc == NSC - 1),
            )

    prev = None
    for sc in range(NSC):
        cur = _mk_t16_t17T(sc)
        if prev is not None:
            _t19_mms(sc - 1, prev)
        prev = cur
    _t19_mms(NSC - 1, prev)

    t19T_sb = [const_pool.tile([PART, C], F32, tag=f"t19T{mc}", name=f"t19T{mc}")
               for mc in range(NCC)]
    for mc in range(NCC):
        nc.scalar.mul(t19T_sb[mc][:], t19_ps[mc][:], S_HW)
    P.close("t17Tp")
    P.close("t16p")
    P.close("p16p")
    P.close("t17p")

    # ----------------- S' = s_c * colsum(t19T) ; A_i = w12n_i (*) t19T -----------------
    s_sb = const_pool.tile([1, C], F32, tag="scol", name="scol")
    psum_s = ps_sm.tile([1, C], F32, tag="onebank", name="onebank")
    for kb in range(NCC):
        nc.tensor.matmul(psum_s[:, :], ones[:], t19T_sb[kb][:],
                         start=(kb == 0), stop=(kb == NCC - 1))
    nc.scalar.mul(s_sb[:], psum_s[:], S_C)

    a_sb = [[const_pool.tile([PART, C], F32, tag=f"a{i}_{dc}", name=f"a{i}_{dc}")
             for dc in range(NCC)] for i in range(3)]
    for i in range(3):
        for dc in range(NCC):
            nc.vector.tensor_scalar_mul(a_sb[i][dc][:], t19T_sb[dc][:],
                                        w12_sb[dc][:, i:i + 1])

    # ----------------- t20 (PSUM) ; out = t20 + t18 -----------------
    out_pool = P.open("outp", bufs=1, side="left")
    out_sb = [out_pool.tile([PART, S], F32, tag=f"out{cc}", name=f"out{cc}")
              for cc in range(NCC)]
    for mc in range(NCC):
        for hc in range(NHC):
            h0 = hc * HCH
            psum = ps_mm.tile([PART, HCH * W], F32, tag="mmbank", name="mmbank")
            nc.tensor.matmul(
                psum[:, :], s_sb[:, mc * PART:(mc + 1) * PART],
                t11_sb[:, h0 * W:(h0 + HCH) * W],
                start=True, stop=False, skip_group_check=True,
            )
            n_parts = []
            for i in range(3):
                dh = 2 * (i - 1)
                hlo = max(h0, -dh)
                hhi = min(h0 + HCH, H - dh)
                if hlo < hhi:
                    n_parts.append((i, dh, hlo, hhi))
            for pi, (i, dh, hlo, hhi) in enumerate(n_parts):
                for dc in range(NCC):
                    nc.tensor.matmul(
                        psum[:, (hlo - h0) * W:(hhi - h0) * W],
                        a_sb[i][dc][:, mc * PART:(mc + 1) * PART],
                        t10_sb[dc][:, (hlo + dh) * W:(hhi + dh) * W],
                        start=False,
                        stop=(pi == len(n_parts) - 1 and dc == NCC - 1),
                        skip_group_check=True,
                    )
            nc.vector.tensor_add(out_sb[mc][:, h0 * W:(h0 + HCH) * W],
                                 t18_sb[mc][:, h0 * W:(h0 + HCH) * W], psum[:])

    out_dv = out_d.rearrange("(k p) s -> k p s", p=PART)
    for cc in range(NCC):
        nc.sync.dma_start(out_dv[cc], out_sb[cc][:])

    P.close_all()


_NC_CACHE = None


def _get_module():
    global _NC_CACHE
    if _NC_CACHE is None:
        _NC_CACHE = build_module()
    return _NC_CACHE


def prep_params(p2, w6, w7, p9, p11, w12, w15, p16):
    p2s = np.ascontiguousarray(np.asarray(p2, np.float32).reshape(C, S))
    w6T = np.ascontiguousarray(np.asarray(w6, np.float32).T)              # [C, C8]
    w7r = np.asarray(w7, np.float32).reshape(C, C, 9).transpose(2, 1, 0)  # [ij, c, o]
    w7r = np.ascontiguousarray(w7r).reshape(9 * C, C)
    p9s = np.ascontiguousarray(np.asarray(p9, np.float32).reshape(C8, S))
    p11a = np.ascontiguousarray(np.asarray(p11, np.float32).reshape(C8, 1))
    w12n = np.ascontiguousarray(
        -S_C * np.asarray(w12, np.float32).reshape(C, 3))                 # [C, 3]
    w15T = np.ascontiguousarray(np.asarray(w15, np.float32).T)            # [c, o]
    p16a = np.ascontiguousarray(np.asarray(p16, np.float32).reshape(C, S))
    return dict(p2s=p2s, w6T=w6T, w7r=w7r, p9s=p9s, p11=p11a, w12n=w12n,
                w15T=w15T, p16=p16a)


def kernel(x, p2, w6, w7, p9, p11, w12, w15, p16):
    nc = _get_module()
    params = prep_params(p2, w6, w7, p9, p11, w12, w15, p16)
    xa = np.ascontiguousarray(np.asarray(x, np.float32).reshape(N, C, S))
    in_maps = [{"x": xa[n], **params} for n in range(NCORES)]
    res = bass_utils.run_bass_kernel_spmd(nc, in_maps, core_ids=list(range(NCORES)))
    out = np.stack([res.results[n]["out"].reshape(C, H, W) for n in range(NCORES)])
    return out
